# revision 30
# baseline (speedup 1.0000x reference)
"""Trainium2 Bass kernel for a 2-layer dual-branch GCN (nn_ATACGCN).

reference:
    zs, zu, za = split(z)
    ys = elu(adj @ (elu(zs) @ W0) + b0); ys = elu(adj @ (ys @ W1) + b1)
    yu = elu(adj @ (elu(zu) @ W0) + b0); yu = elu(adj @ (yu @ W1) + b1)
    out = concat(ys, yu, za) @ Wl + bl

Strategy: 1D row-shard of the node dimension across 8 NeuronCores. Both
branches share weights, so they are fused into one 128-wide feature block
(block-diagonal W). Each core computes Y^T = H^T @ adjT in PSUM, streaming
its [16384, 2048] slab of adj^T (fp8, host-scaled by N) from HBM.

The kernel is DMA-bound (adj streaming), so the first B_RES kb-blocks of the
adj slab are kept RESIDENT in SBUF across both GCN layers -- layer 2 only
re-streams the non-resident remainder. Stage A (input elu + H0 = E @ W0) is
sharded across cores and AllGathered, like the inter-layer exchange, instead
of being computed redundantly on every core. h-path DMAs ride the ACT HWDGE
ring so they never queue behind bulk adj DMAs on the SP ring.

elu(x) is composed as min(exp(x) - 1, max(x, 0)); exp/relu run on ACT with
the fp8 descale (1/n) and layer bias folded into the activation's
scale/bias operands.
"""

import numpy as np
import ml_dtypes

BF16 = ml_dtypes.bfloat16
FP8 = ml_dtypes.float8_e4m3

# Problem constants (hardcoded per harness contract).
N = 16384      # nodes
D = 64         # per-branch width
OUT = 64       # output width
L = 2          # gcn layers
N_CORES = 8
P = 128        # SBUF partitions
RPC = N // N_CORES          # rows (nodes) per core

# Tunables
B_RES = 32                  # resident kb-blocks (of kt//2 = 64); 4 KiB/part each
RES_GRP = 4                 # kb-blocks per resident group DMA (2 MiB DMAs)
SG = 2                      # kb-blocks per streamed slab DMA (1 MiB DMAs)
RING1 = 2                   # L1 streaming slab ring depth (units of SG kbs)
RING2 = 2                   # L2 streaming slab ring depth (fresh tag: prefetch
                            # across the inter-layer collective)
CW_SUB = 512                # stage-A / elu subchunk width
NCHUNK = 1                  # inter-layer AllGather chunks (pipelined)
GATHER_KIND = "Tree"        # "AllGather" (ring, ~10us/step x 7),
                            # "AllToAll" (input replicated 8x to emulate AG),
                            # "Tree" (3 rounds of pairwise AllGathers)


def build_kernel_body(tc, ins, outs, n_cores=N_CORES, n=N, with_collective=True,
                      b_res=B_RES, ring1=RING1, ring2=RING2):
    """Emit the per-core Tile program (fp8 adj + fp8 H, DoubleRow matmuls).

    ins/outs: dicts name -> bass.AP of the DRAM I/O tensors:
      adjt [n, rpc] fp8 (adj.T * n, host-scaled), zsut [128, rpc] bf16,
      zat [64, rpc] bf16, wbd [128, 2*128] bf16, wlsu [128, 64] bf16,
      wlza [64, 64] bf16, blr [1, 64] bf16, bias [128, 2] f32
      ->  outp [rpc, 64] f32
    """
    import concourse.mybir as mybir

    nc = tc.nc
    dt = mybir.dt
    f32, bf = dt.float32, dt.bfloat16
    AF = mybir.ActivationFunctionType
    ALU = mybir.AluOpType
    adt = dt.float8e4
    hdt = dt.float8e4
    inv_n = 1.0 / n

    rpc = n // n_cores
    kt = n // P                  # 128 k-tiles
    nkb = kt // 2                # 64 kb-blocks (DoubleRow: 2 k-tiles each)
    t_pc = rpc // P              # 16 node tiles per core
    cw_y = 512                   # PSUM chunk width (one f32 bank)
    nch_y = rpc // cw_y          # 4
    n_sub = rpc // CW_SUB        # stage-A subchunks
    assert b_res % RES_GRP == 0
    n_res_grp = b_res // RES_GRP

    adjt = ins["adjt"]
    zsut = ins["zsut"]
    zat = ins["zat"]
    wbd, wlsu, wlza = ins["wbd"], ins["wlsu"], ins["wlza"]
    blr, bias = ins["blr"], ins["bias"]
    outp = outs["outp"]

    # DRAM view: q = global k-tile index (0..127).
    adjq = adjt.rearrange("(q p) m -> p q m", p=P)   # [128, 128, rpc]

    with (
        tc.tile_pool(name="consts", bufs=1) as consts,
        tc.tile_pool(name="respool", bufs=1) as respool,
        tc.tile_pool(name="hpool", bufs=1) as hpool,
        tc.tile_pool(name="adjp", bufs=ring1) as adjp,
        tc.tile_pool(name="adjp2", bufs=ring2) as adjp2,
        tc.tile_pool(name="tmp", bufs=2) as tmp,
        tc.tile_pool(name="xp", bufs=1) as xp,
        tc.tile_pool(name="ps", bufs=1, space="PSUM") as ps,
        tc.tile_pool(name="dram", bufs=1, space="DRAM") as dram,
    ):
        # ---- constants to SBUF ----
        wbd_sb = consts.tile([P, L * P], bf, name="wbd_sb")
        nc.scalar.dma_start(out=wbd_sb[:], in_=wbd[:])
        wlsu_sb = consts.tile([P, OUT], bf, name="wlsu_sb")
        nc.scalar.dma_start(out=wlsu_sb[:], in_=wlsu[:])
        wlza_sb = consts.tile([D, OUT], bf, name="wlza_sb")
        nc.scalar.dma_start(out=wlza_sb[:], in_=wlza[:])
        blr_sb = consts.tile([1, OUT], bf, name="blr_sb")
        nc.scalar.dma_start(out=blr_sb[:], in_=blr[:])
        bias_sb = consts.tile([P, L], f32, name="bias_sb")
        nc.scalar.dma_start(out=bias_sb[:], in_=bias[:])
        zat_sb = consts.tile([D, rpc], bf, name="zat_sb")
        nc.sync.dma_start(out=zat_sb[:], in_=zat[:])
        ones_sb = consts.tile([1, P], bf, name="ones_sb")
        nc.vector.memset(ones_sb[:], 1.0)

        # Persistent H tile (shared between layers; 16 KiB/partition).
        # Layout: h[p, q*128 + f] = H[node q*128+p, f].
        h = hpool.tile([P, n], hdt, name="h", tag="h")
        hq = h.rearrange("p (q f) -> p q f", f=P)

        def emit_gather_chunk(hm, c, rest_order):
            """AllGather chunk c of hm [P, rpc] into h (all cores' blocks).

            g_in rows are ordered (partition, tile-within-chunk) so that the
            g_in write and the h restage run with >=1 KiB-contiguous
            descriptors per partition (128 B descriptors otherwise -- far
            below the 512 B full-rate SDMA minimum).
            """
            wc = rpc // NCHUNK          # hm cols per chunk
            if GATHER_KIND == "Tree":
                # Recursive-doubling AllGather: 3 rounds of pairwise
                # exchanges (1 ring step each) instead of one 7-step ring --
                # the ~10us/step ncfw control floor dominates at this size.
                g_in = dram.tile([wc, P], hdt, name=f"g_in{c}")
                nc.scalar.dma_start(
                    out=g_in.rearrange("(p t) f -> p (t f)", p=P),
                    in_=hm[:, c * wc:(c + 1) * wc],
                )
                rounds = [
                    [[2 * a, 2 * a + 1] for a in range(4)],
                    [[0, 2], [1, 3], [4, 6], [5, 7]],
                    [[0, 4], [1, 5], [2, 6], [3, 7]],
                ]
                cur = g_in
                for r, groups in enumerate(rounds):
                    nxt = dram.tile([wc << (r + 1), P], hdt,
                                    name=f"g_t{c}_{r}")
                    if with_collective and n_cores > 1:
                        nc.gpsimd.collective_compute(
                            "AllGather",
                            mybir.AluOpType.bypass,
                            replica_groups=groups,
                            ins=[cur.opt()],
                            outs=[nxt.opt()],
                        )
                    else:
                        nc.scalar.dma_start(
                            out=nxt[:wc << r, :], in_=cur[:])
                    cur = nxt
                g_out = cur
                gm = g_out.rearrange("(m p w) f -> p m (w f)", m=n_cores,
                                     p=P)
                hv = h.rearrange("p (m c w) -> p m c w", m=n_cores, c=NCHUNK)
                for g in rest_order:
                    nc.scalar.dma_start(
                        out=hv[:, g:g + 1, c, :],
                        in_=gm[:, g:g + 1, :],
                    )
                return
            if GATHER_KIND == "AllToAll":
                # A2A with the input replicated n_cores x emulates AllGather
                # with direct peer sends instead of a 7-step ring.
                g_in = dram.tile([n // NCHUNK, P], hdt, name=f"g_in{c}")
                grep = g_in.rearrange("(m p t) f -> m p (t f)", m=n_cores,
                                      p=P)
                for j in range(n_cores):
                    nc.scalar.dma_start(
                        out=grep[j], in_=hm[:, c * wc:(c + 1) * wc])
            else:
                g_in = dram.tile([wc, P], hdt, name=f"g_in{c}")
                nc.scalar.dma_start(
                    out=g_in.rearrange("(p t) f -> p (t f)", p=P),
                    in_=hm[:, c * wc:(c + 1) * wc],
                )
            if with_collective and n_cores > 1:
                g_out = dram.tile(
                    [n // NCHUNK, P], hdt, name=f"g_out{c}",
                    addr_space="Shared" if GATHER_KIND == "AllGather" else "Local",
                )
                nc.gpsimd.collective_compute(
                    GATHER_KIND,
                    mybir.AluOpType.bypass,
                    replica_groups=[list(range(n_cores))],
                    ins=[g_in.opt()],
                    outs=[g_out.opt()],
                )
            else:
                # cost-model-only path (TimelineSim): same DMA pattern minus
                # the collective. Numerically invalid for other cores' tiles.
                g_out = dram.tile([n // NCHUNK, P], hdt, name=f"g_out{c}")
                nc.scalar.dma_start(out=g_out[:wc, :], in_=g_in[:wc, :])
            # g_out row m*wc/128 ... (m, p, t) holds H[node m*rpc + (c*tc+t)*128 + p].
            gm = g_out.rearrange("(m p w) f -> p m (w f)", m=n_cores, p=P)
            hv = h.rearrange("p (m c w) -> p m c w", m=n_cores, c=NCHUNK)
            for g in rest_order:
                nc.scalar.dma_start(
                    out=hv[:, g:g + 1, c, :],
                    in_=gm[:, g:g + 1, :],
                )

        # ---- stage A (redundant on every core): H0 = elu(zsu) @ W0bd ----
        # Full recompute instead of shard+AllGather: one fewer collective
        # (~36 us on HW) for ~11 us of extra zsut streaming, fully overlapped
        # with the resident-adj loads at startup.
        zbig = n // 8
        for ch in range(n // zbig):
            zch = tmp.tile([P, zbig], bf, name="zch", tag="zch")
            nc.scalar.dma_start(
                out=zch[:], in_=zsut[:, ch * zbig:(ch + 1) * zbig])
            for sc in range(zbig // CW_SUB):
                sl = slice(sc * CW_SUB, (sc + 1) * CW_SUB)
                hsl = slice(ch * zbig + sc * CW_SUB,
                            ch * zbig + (sc + 1) * CW_SUB)
                e_t = tmp.tile([P, CW_SUB], f32, name="e_t", tag="e_t")
                nc.scalar.activation(e_t[:], zch[:, sl], AF.Exp)
                m_t = tmp.tile([P, CW_SUB], f32, name="m_t", tag="m_t")
                nc.scalar.activation(m_t[:], zch[:, sl], AF.Relu)
                x0 = tmp.tile([P, CW_SUB], bf, name="x0", tag="x0")
                nc.vector.scalar_tensor_tensor(
                    x0[:], e_t[:], -1.0, m_t[:], op0=ALU.add, op1=ALU.min
                )
                ph = ps.tile([P, CW_SUB], f32, name="ph", tag="ph", bufs=2)
                for t4 in range(CW_SUB // P):
                    nc.tensor.matmul(
                        ph[:, t4 * P:(t4 + 1) * P],
                        lhsT=x0[:, t4 * P:(t4 + 1) * P], rhs=wbd_sb[:, 0:P],
                        start=True, stop=True,
                    )
                nc.vector.tensor_copy(h[:, hsl], ph[:])

        # ---- resident adj groups (loaded once, used by both layers) ----
        res_tiles = [
            respool.tile([P, RES_GRP * 2, rpc], adt, name=f"res{g}",
                         tag=f"res{g}")
            for g in range(n_res_grp)
        ]
        # L1's last ring1 slab units stay valid in SBUF; L2 reuses them.
        l1_tail = {}     # unit kb0 -> slab tile

        # Work units: ("res", g) = resident group (RES_GRP kbs, no DMA in L2),
        # ("slab", kb0) = streamed slab of SG kbs, ("tail", kb0) = L1-tail
        # slab reused in L2 without DMA.
        s_units = list(range(b_res, nkb, SG))
        tail_units = s_units[len(s_units) - ring1:]
        res_units = [("res", g) for g in range(n_res_grp)]

        # L1: residents first (PE races ahead while their 2 MiB loads and the
        # stream pipeline fill DMA), streamed units last (DMA-paced finish).
        l1_order = res_units + [("slab", kb0) for kb0 in s_units]

        # L2, per gather chunk (chunk c covers kbs with kb%8 in its band):
        # ring2-prefetched stream units first (release stream slots the
        # moment the chunk's h lands), then remaining stream units
        # Bresenham-interleaved with no-DMA units (residents + reused
        # L1-tail slabs) so PE slot-release tracks DMA pace and resident
        # work fills the DMA-bound phase instead of forming a serial PE tail.
        def unit_kbs(u):
            kind, v = u
            if kind == "res":
                return list(range(v * RES_GRP, (v + 1) * RES_GRP))
            return list(range(v, v + SG))

        band = (nkb // n_cores) // NCHUNK        # kb%8 band width per chunk

        def unit_chunk(u):
            kbs = unit_kbs(u)
            cs = {(kb % (nkb // n_cores)) // band for kb in kbs}
            assert len(cs) == 1, f"unit {u} straddles gather chunks"
            return cs.pop()

        all_units = (res_units
                     + [("tail", kb0) for kb0 in tail_units]
                     + [("slab", kb0) for kb0 in s_units
                        if kb0 not in tail_units])
        l2_order = []
        for c in range(NCHUNK):
            cu = [u for u in all_units if unit_chunk(u) == c]
            streams = [u for u in cu if u[0] == "slab"]
            nodma = [u for u in cu if u[0] != "slab"]
            nodma_kbs = sum(len(unit_kbs(u)) for u in nodma)
            pre = streams[:ring2] if c == 0 else []
            rest_s = streams[len(pre):]
            l2_order.extend(pre)
            ratio = nodma_kbs / max(1, len(rest_s))
            acc, ri = 0.0, 0
            for u in rest_s:
                l2_order.append(u)
                acc += ratio
                while acc >= 1.0 and ri < len(nodma):
                    l2_order.append(nodma[ri])
                    acc -= len(unit_kbs(nodma[ri]))
                    ri += 1
            l2_order.extend(nodma[ri:])

        assert sorted(kb for u in l2_order for kb in unit_kbs(u)) == \
            list(range(nkb))
        # per-chunk restage-DMA order: deliver h m-groups in consumption order
        rest2 = [[] for _ in range(NCHUNK)]
        seen = set()
        for u in l2_order:
            for kb in unit_kbs(u):
                m = kb // (nkb // n_cores)
                c = unit_chunk(u)
                if (m, c) not in seen:
                    seen.add((m, c))
                    rest2[c].append(m)
        unit_orders = [l1_order, l2_order]

        for layer in range(L):
            # ---- big matmul: Y^T[feat, local nodes] = H^T @ adjT ----
            psy = [
                ps.tile([P, cw_y], f32, name=f"psy{c}", tag=f"psy{c}", bufs=1)
                for c in range(nch_y)
            ]
            pos = 0
            for u in unit_orders[layer]:
                kind, v = u
                if kind == "res":
                    if layer == 0:
                        nc.sync.dma_start(
                            out=res_tiles[v][:],
                            in_=adjq[:, v * RES_GRP * 2:(v + 1) * RES_GRP * 2, :],
                        )
                    rhs_t = res_tiles[v]
                    kb0 = v * RES_GRP
                elif kind == "tail" or (layer == 1 and v in l1_tail):
                    rhs_t = l1_tail[v]
                    kb0 = v
                else:
                    pool = adjp if layer == 0 else adjp2
                    slab = pool.tile([P, 2 * SG, rpc], adt, name="slab",
                                     tag=f"slab{layer}")
                    nc.sync.dma_start(
                        out=slab[:], in_=adjq[:, 2 * v:2 * (v + SG), :]
                    )
                    if layer == 0 and v in tail_units:
                        l1_tail[v] = slab
                    rhs_t = slab
                    kb0 = v
                for i, kb in enumerate(unit_kbs(u)):
                    rhs3 = rhs_t[:, 2 * i:2 * i + 2, :]
                    lhsT = hq[:, 2 * kb:2 * kb + 2, :]
                    for c in range(nch_y):
                        nc.tensor.matmul(
                            psy[c][:],
                            lhsT=lhsT,
                            rhs=rhs3[:, :, c * cw_y:(c + 1) * cw_y],
                            start=(pos == 0), stop=(pos == nkb - 1),
                            perf_mode=mybir.MatmulPerfMode.DoubleRow,
                        )
                    pos += 1

            # ---- per-chunk: X^T = elu(Y^T/n + b), then H1 | final out ----
            b_ap = bias_sb[:, layer:layer + 1]
            hm1 = None
            if layer < L - 1:
                hm1 = xp.tile([P, rpc], hdt, name="hm", tag="hm")
            gather_after = {
                (c2 + 1) * (nch_y // NCHUNK) - 1: c2 for c2 in range(NCHUNK)
            }
            for c in range(nch_y):
                e_t = tmp.tile([P, cw_y], f32, name="e_t", tag="e_t")
                nc.scalar.activation(e_t[:], psy[c][:], AF.Exp,
                                     bias=b_ap, scale=inv_n)
                m_t = tmp.tile([P, cw_y], f32, name="m_t", tag="m_t")
                nc.scalar.activation(m_t[:], psy[c][:], AF.Relu,
                                     bias=b_ap, scale=inv_n)
                xc = tmp.tile([P, cw_y], bf, name="xc", tag="xc")
                nc.vector.scalar_tensor_tensor(
                    xc[:], e_t[:], -1.0, m_t[:], op0=ALU.add, op1=ALU.min,
                )
                if layer < L - 1:
                    # H1 chunk: 4 node tiles -> one PSUM bank -> hm1
                    ph = ps.tile([P, 4 * P], f32, name="ph", tag="ph", bufs=2)
                    for t4 in range(4):
                        nc.tensor.matmul(
                            ph[:, t4 * P:(t4 + 1) * P],
                            lhsT=xc[:, t4 * P:(t4 + 1) * P],
                            rhs=wbd_sb[:, P:2 * P], start=True, stop=True,
                        )
                    nc.vector.tensor_copy(
                        hm1[:, c * cw_y:(c + 1) * cw_y], ph[:])
                    if c in gather_after:
                        c2 = gather_after[c]
                        emit_gather_chunk(hm1, c2, rest2[c2])
                else:
                    # final: out = [ys yu] @ Wl[:128] + za @ Wl[128:] + bl
                    po = ps.tile([P, 4 * OUT], f32, name="po", tag="po",
                                 bufs=2)
                    for t4 in range(4):
                        t = c * 4 + t4
                        osl = slice(t4 * OUT, (t4 + 1) * OUT)
                        nc.tensor.matmul(
                            po[:, osl], lhsT=xc[:, t4 * P:(t4 + 1) * P],
                            rhs=wlsu_sb[:],
                            start=True, stop=False, skip_group_check=True,
                        )
                        nc.tensor.matmul(
                            po[:, osl], lhsT=zat_sb[:, t * P:(t + 1) * P],
                            rhs=wlza_sb[:],
                            start=False, stop=False, skip_group_check=True,
                        )
                        nc.tensor.matmul(
                            po[:, osl], lhsT=ones_sb[:], rhs=blr_sb[:],
                            start=False, stop=True, skip_group_check=True,
                        )
                    ot = tmp.tile([P, 4 * OUT], f32, name="ot", tag="ot")
                    nc.vector.tensor_copy(ot[:], po[:])
                    nc.sync.dma_start(
                        out=outp[c * cw_y:(c + 1) * cw_y, :].rearrange(
                            "(t p) f -> p t f", p=P),
                        in_=ot.rearrange("p (t f) -> p t f", f=OUT),
                    )


def build_full(n_cores=N_CORES, n=N, adj_dtype="fp8", h_dtype="fp8",
               use_doublerow=True, num_devices=None, with_collective=True,
               repeats=1, b_res=B_RES, ring1=RING1, ring2=RING2):
    """Build + compile the full SPMD Bass module (one program, 8 cores)."""
    import concourse.bacc as bacc
    import concourse.mybir as mybir
    import concourse.tile as tile

    dt = mybir.dt
    f32, bf = dt.float32, dt.bfloat16
    adt = dt.float8e4
    rpc = n // n_cores
    if num_devices is None:
        num_devices = n_cores

    nc = bacc.Bacc("TRN2", target_bir_lowering=False, debug=False,
                   num_devices=num_devices)
    ins = {
        "adjt": nc.dram_tensor("adjt", [n, rpc], adt, kind="ExternalInput").ap(),
        "zsut": nc.dram_tensor("zsut", [P, n], bf, kind="ExternalInput").ap(),
        "zat": nc.dram_tensor("zat", [D, rpc], bf, kind="ExternalInput").ap(),
        "wbd": nc.dram_tensor("wbd", [P, L * P], bf, kind="ExternalInput").ap(),
        "wlsu": nc.dram_tensor("wlsu", [P, OUT], bf, kind="ExternalInput").ap(),
        "wlza": nc.dram_tensor("wlza", [D, OUT], bf, kind="ExternalInput").ap(),
        "blr": nc.dram_tensor("blr", [1, OUT], bf, kind="ExternalInput").ap(),
        "bias": nc.dram_tensor("bias", [P, L], f32, kind="ExternalInput").ap(),
    }
    outs = {
        "outp": nc.dram_tensor("outp", [rpc, OUT], f32, kind="ExternalOutput").ap(),
    }
    with tile.TileContext(nc) as tc:
        for _ in range(repeats):
            build_kernel_body(tc, ins, outs, n_cores=n_cores, n=n,
                              with_collective=with_collective, b_res=b_res,
                              ring1=ring1, ring2=ring2)
    nc.compile()
    return nc


def prep_inputs(z, adj, Ws, bs, Wl, bl, n_cores=N_CORES, n=N, adj_dtype="fp8"):
    """Host-side sharding: build the per-core input maps."""
    rpc = n // n_cores
    z = np.asarray(z, dtype=np.float32)
    adj = np.asarray(adj, dtype=np.float32)
    Ws = np.asarray(Ws, dtype=np.float32)
    bs = np.asarray(bs, dtype=np.float32)
    Wl = np.asarray(Wl, dtype=np.float32)
    bl = np.asarray(bl, dtype=np.float32)

    adjt = (adj.T * np.float32(n)).astype(FP8)           # [n, n] in [0,1)
    zsut = np.ascontiguousarray(z[:, :2 * D].T).astype(BF16)  # [128, n]
    zat = np.ascontiguousarray(z[:, 2 * D:].T).astype(BF16)   # [64, n]

    wbd = np.zeros((P, L * P), dtype=np.float32)
    for l in range(L):
        wbd[:D, l * P:l * P + D] = Ws[l]
        wbd[D:, l * P + D:(l + 1) * P] = Ws[l]
    wbd = wbd.astype(BF16)
    bias = np.stack([np.concatenate([bs[l], bs[l]]) for l in range(L)],
                    axis=1).astype(np.float32)           # [128, L]
    wlsu = np.ascontiguousarray(Wl[:2 * D]).astype(BF16)
    wlza = np.ascontiguousarray(Wl[2 * D:]).astype(BF16)
    blr = np.ascontiguousarray(bl.reshape(1, OUT)).astype(BF16)

    in_maps = []
    for m in range(n_cores):
        sl = slice(m * rpc, (m + 1) * rpc)
        in_maps.append({
            "adjt": np.ascontiguousarray(adjt[:, sl]),
            "zsut": zsut,
            "zat": np.ascontiguousarray(zat[:, sl]),
            "wbd": wbd,
            "wlsu": wlsu,
            "wlza": wlza,
            "blr": blr,
            "bias": bias,
        })
    return in_maps


_NC_CACHE = {}
ADJ_DTYPE = "fp8"
H_DTYPE = "fp8"
USE_DOUBLEROW = True


def kernel(z, adj, Ws, bs, Wl, bl):
    """Full-input entry point: shard, run on 8 NeuronCores, gather."""
    from concourse.bass_utils import run_bass_kernel_spmd

    if "nc" not in _NC_CACHE:
        _NC_CACHE["nc"] = build_full()
    nc = _NC_CACHE["nc"]

    in_maps = prep_inputs(z, adj, Ws, bs, Wl, bl)
    res = run_bass_kernel_spmd(nc, in_maps, core_ids=list(range(N_CORES)))
    out = np.concatenate(
        [res.results[m]["outp"] for m in range(N_CORES)], axis=0
    ).astype(np.float32)
    return out


# revision 32
# speedup vs baseline: 1.1847x; 1.1847x over previous
"""Trainium2 Bass kernel for a 2-layer dual-branch GCN (nn_ATACGCN).

reference:
    zs, zu, za = split(z)
    ys = elu(adj @ (elu(zs) @ W0) + b0); ys = elu(adj @ (ys @ W1) + b1)
    yu = elu(adj @ (elu(zu) @ W0) + b0); yu = elu(adj @ (yu @ W1) + b1)
    out = concat(ys, yu, za) @ Wl + bl

Strategy: 1D row-shard of the node dimension across 8 NeuronCores. Both
branches share weights, so they are fused into one 128-wide feature block
(block-diagonal W). Each core computes Y^T = H^T @ adjT in PSUM, streaming
its [16384, 2048] slab of adj^T (fp8, host-scaled by N) from HBM.

The kernel is DMA-bound (adj streaming), so the first B_RES kb-blocks of the
adj slab are kept RESIDENT in SBUF across both GCN layers -- layer 2 only
re-streams the non-resident remainder, reuses L1's last ring slabs straight
from SBUF, and interleaves the no-DMA (resident) work with the re-streamed
work so PE-paced slot release tracks DMA pace. Stage A (input elu + H0) is
computed redundantly on every core (one AllGather is ~30-100 us of ncfw
overhead on HW -- the single inter-layer AllGather is kept, its restage
ordered to deliver h in L2's consumption order). h-path DMAs ride the ACT
HWDGE ring so they never queue behind bulk adj DMAs on the SP ring, and the
gather staging buffers use a (partition, tile) row order so every descriptor
is >=1 KiB-contiguous (128 B descriptors are far below SDMA line rate).

elu(x) is composed as min(exp(x) - 1, max(x, 0)); exp/relu run on ACT with
the fp8 descale (1/n) and layer bias folded into the activation's
scale/bias operands.
"""

import numpy as np
import ml_dtypes

BF16 = ml_dtypes.bfloat16
FP8 = ml_dtypes.float8_e4m3

# Problem constants (hardcoded per harness contract).
N = 16384      # nodes
D = 64         # per-branch width
OUT = 64       # output width
L = 2          # gcn layers
N_CORES = 8
P = 128        # SBUF partitions
RPC = N // N_CORES          # rows (nodes) per core

# Tunables
B_RES = 32                  # resident kb-blocks (of kt//2 = 64); 4 KiB/part each
RES_GRP = 4                 # kb-blocks per resident group DMA (2 MiB DMAs)
SG = 2                      # kb-blocks per streamed slab DMA (1 MiB DMAs)
RING1 = 2                   # L1 streaming slab ring depth (units of SG kbs)
RING2 = 2                   # L2 streaming slab ring depth (fresh tag: prefetch
                            # across the inter-layer collective)
CW_SUB = 512                # stage-A / elu subchunk width
NCHUNK = 1                  # inter-layer AllGather chunks (pipelined)
GATHER_KIND = "AllGather"   # "AllGather" (ring, ~10us/step x 7),
                            # "AllToAll" (input replicated 8x to emulate AG),
                            # "Tree" (3 rounds of pairwise AllGathers)


def build_kernel_body(tc, ins, outs, n_cores=N_CORES, n=N, with_collective=True,
                      b_res=B_RES, ring1=RING1, ring2=RING2):
    """Emit the per-core Tile program (fp8 adj + fp8 H, DoubleRow matmuls).

    ins/outs: dicts name -> bass.AP of the DRAM I/O tensors:
      adjt [n, rpc] fp8 (adj.T * n, host-scaled), zsut [128, rpc] bf16,
      zat [64, rpc] bf16, wbd [128, 2*128] bf16, wlsu [128, 64] bf16,
      wlza [64, 64] bf16, blr [1, 64] bf16, bias [128, 2] f32
      ->  outp [rpc, 64] f32
    """
    import concourse.mybir as mybir

    nc = tc.nc
    dt = mybir.dt
    f32, bf = dt.float32, dt.bfloat16
    AF = mybir.ActivationFunctionType
    ALU = mybir.AluOpType
    adt = dt.float8e4
    hdt = dt.float8e4
    inv_n = 1.0 / n

    rpc = n // n_cores
    kt = n // P                  # 128 k-tiles
    nkb = kt // 2                # 64 kb-blocks (DoubleRow: 2 k-tiles each)
    t_pc = rpc // P              # 16 node tiles per core
    cw_y = 512                   # PSUM chunk width (one f32 bank)
    nch_y = rpc // cw_y          # 4
    n_sub = rpc // CW_SUB        # stage-A subchunks
    assert b_res % RES_GRP == 0
    n_res_grp = b_res // RES_GRP

    adjt = ins["adjt"]
    zsut = ins["zsut"]
    zat = ins["zat"]
    wbd, wlsu, wlza = ins["wbd"], ins["wlsu"], ins["wlza"]
    blr, bias = ins["blr"], ins["bias"]
    outp = outs["outp"]

    # DRAM view: q = global k-tile index (0..127).
    adjq = adjt.rearrange("(q p) m -> p q m", p=P)   # [128, 128, rpc]

    with (
        tc.tile_pool(name="consts", bufs=1) as consts,
        tc.tile_pool(name="respool", bufs=1) as respool,
        tc.tile_pool(name="hpool", bufs=1) as hpool,
        tc.tile_pool(name="adjp", bufs=ring1) as adjp,
        tc.tile_pool(name="adjp2", bufs=ring2) as adjp2,
        tc.tile_pool(name="tmp", bufs=2) as tmp,
        tc.tile_pool(name="xp", bufs=1) as xp,
        tc.tile_pool(name="ps", bufs=1, space="PSUM") as ps,
        tc.tile_pool(name="dram", bufs=1, space="DRAM") as dram,
    ):
        # ---- constants to SBUF ----
        wbd_sb = consts.tile([P, L * P], bf, name="wbd_sb")
        nc.scalar.dma_start(out=wbd_sb[:], in_=wbd[:])
        wlsu_sb = consts.tile([P, OUT], bf, name="wlsu_sb")
        nc.scalar.dma_start(out=wlsu_sb[:], in_=wlsu[:])
        wlza_sb = consts.tile([D, OUT], bf, name="wlza_sb")
        nc.scalar.dma_start(out=wlza_sb[:], in_=wlza[:])
        blr_sb = consts.tile([1, OUT], bf, name="blr_sb")
        nc.scalar.dma_start(out=blr_sb[:], in_=blr[:])
        bias_sb = consts.tile([P, L], f32, name="bias_sb")
        nc.scalar.dma_start(out=bias_sb[:], in_=bias[:])
        zat_sb = consts.tile([D, rpc], bf, name="zat_sb")
        nc.sync.dma_start(out=zat_sb[:], in_=zat[:])
        ones_sb = consts.tile([1, P], bf, name="ones_sb")
        nc.vector.memset(ones_sb[:], 1.0)

        # Persistent H tile (shared between layers; 16 KiB/partition).
        # Layout: h[p, q*128 + f] = H[node q*128+p, f].
        h = hpool.tile([P, n], hdt, name="h", tag="h")
        hq = h.rearrange("p (q f) -> p q f", f=P)

        def emit_gather_chunk(hm, c, rest_order):
            """AllGather chunk c of hm [P, rpc] into h (all cores' blocks).

            g_in rows are ordered (partition, tile-within-chunk) so that the
            g_in write and the h restage run with >=1 KiB-contiguous
            descriptors per partition (128 B descriptors otherwise -- far
            below the 512 B full-rate SDMA minimum).
            """
            wc = rpc // NCHUNK          # hm cols per chunk
            if GATHER_KIND == "Tree":
                # Recursive-doubling AllGather: 3 rounds of pairwise
                # exchanges (1 ring step each) instead of one 7-step ring --
                # the ~10us/step ncfw control floor dominates at this size.
                g_in = dram.tile([wc, P], hdt, name=f"g_in{c}")
                nc.scalar.dma_start(
                    out=g_in.rearrange("(p t) f -> p (t f)", p=P),
                    in_=hm[:, c * wc:(c + 1) * wc],
                )
                rounds = [
                    [[2 * a, 2 * a + 1] for a in range(4)],
                    [[0, 2], [1, 3], [4, 6], [5, 7]],
                    [[0, 4], [1, 5], [2, 6], [3, 7]],
                ]
                cur = g_in
                for r, groups in enumerate(rounds):
                    nxt = dram.tile([wc << (r + 1), P], hdt,
                                    name=f"g_t{c}_{r}")
                    if with_collective and n_cores > 1:
                        nc.gpsimd.collective_compute(
                            "AllGather",
                            mybir.AluOpType.bypass,
                            replica_groups=groups,
                            ins=[cur.opt()],
                            outs=[nxt.opt()],
                        )
                    else:
                        nc.scalar.dma_start(
                            out=nxt[:wc << r, :], in_=cur[:])
                    cur = nxt
                g_out = cur
                gm = g_out.rearrange("(m p w) f -> p m (w f)", m=n_cores,
                                     p=P)
                hv = h.rearrange("p (m c w) -> p m c w", m=n_cores, c=NCHUNK)
                for g in rest_order:
                    nc.scalar.dma_start(
                        out=hv[:, g:g + 1, c, :],
                        in_=gm[:, g:g + 1, :],
                    )
                return
            if GATHER_KIND == "AllToAll":
                # A2A with the input replicated n_cores x emulates AllGather
                # with direct peer sends instead of a 7-step ring.
                g_in = dram.tile([n // NCHUNK, P], hdt, name=f"g_in{c}")
                grep = g_in.rearrange("(m p t) f -> m p (t f)", m=n_cores,
                                      p=P)
                for j in range(n_cores):
                    nc.scalar.dma_start(
                        out=grep[j], in_=hm[:, c * wc:(c + 1) * wc])
            else:
                g_in = dram.tile([wc, P], hdt, name=f"g_in{c}")
                nc.scalar.dma_start(
                    out=g_in.rearrange("(p t) f -> p (t f)", p=P),
                    in_=hm[:, c * wc:(c + 1) * wc],
                )
            if with_collective and n_cores > 1:
                g_out = dram.tile(
                    [n // NCHUNK, P], hdt, name=f"g_out{c}",
                    addr_space="Shared" if GATHER_KIND == "AllGather" else "Local",
                )
                nc.gpsimd.collective_compute(
                    GATHER_KIND,
                    mybir.AluOpType.bypass,
                    replica_groups=[list(range(n_cores))],
                    ins=[g_in.opt()],
                    outs=[g_out.opt()],
                )
            else:
                # cost-model-only path (TimelineSim): same DMA pattern minus
                # the collective. Numerically invalid for other cores' tiles.
                g_out = dram.tile([n // NCHUNK, P], hdt, name=f"g_out{c}")
                nc.scalar.dma_start(out=g_out[:wc, :], in_=g_in[:wc, :])
            # g_out row m*wc/128 ... (m, p, t) holds H[node m*rpc + (c*tc+t)*128 + p].
            gm = g_out.rearrange("(m p w) f -> p m (w f)", m=n_cores, p=P)
            hv = h.rearrange("p (m c w) -> p m c w", m=n_cores, c=NCHUNK)
            for g in rest_order:
                nc.scalar.dma_start(
                    out=hv[:, g:g + 1, c, :],
                    in_=gm[:, g:g + 1, :],
                )

        # ---- stage A (redundant on every core): H0 = elu(zsu) @ W0bd ----
        # Full recompute instead of shard+AllGather: one fewer collective
        # (~36 us on HW) for ~11 us of extra zsut streaming, fully overlapped
        # with the resident-adj loads at startup.
        zbig = n // 8
        for ch in range(n // zbig):
            zch = tmp.tile([P, zbig], bf, name="zch", tag="zch")
            nc.scalar.dma_start(
                out=zch[:], in_=zsut[:, ch * zbig:(ch + 1) * zbig])
            for sc in range(zbig // CW_SUB):
                sl = slice(sc * CW_SUB, (sc + 1) * CW_SUB)
                hsl = slice(ch * zbig + sc * CW_SUB,
                            ch * zbig + (sc + 1) * CW_SUB)
                e_t = tmp.tile([P, CW_SUB], f32, name="e_t", tag="e_t")
                nc.scalar.activation(e_t[:], zch[:, sl], AF.Exp)
                m_t = tmp.tile([P, CW_SUB], f32, name="m_t", tag="m_t")
                nc.scalar.activation(m_t[:], zch[:, sl], AF.Relu)
                x0 = tmp.tile([P, CW_SUB], bf, name="x0", tag="x0")
                nc.vector.scalar_tensor_tensor(
                    x0[:], e_t[:], -1.0, m_t[:], op0=ALU.add, op1=ALU.min
                )
                ph = ps.tile([P, CW_SUB], f32, name="ph", tag="ph", bufs=2)
                for t4 in range(CW_SUB // P):
                    nc.tensor.matmul(
                        ph[:, t4 * P:(t4 + 1) * P],
                        lhsT=x0[:, t4 * P:(t4 + 1) * P], rhs=wbd_sb[:, 0:P],
                        start=True, stop=True,
                    )
                nc.vector.tensor_copy(h[:, hsl], ph[:])

        # ---- resident adj groups (loaded once, used by both layers) ----
        res_tiles = [
            respool.tile([P, RES_GRP * 2, rpc], adt, name=f"res{g}",
                         tag=f"res{g}")
            for g in range(n_res_grp)
        ]
        # L1's last ring1 slab units stay valid in SBUF; L2 reuses them.
        l1_tail = {}     # unit kb0 -> slab tile

        # Work units: ("res", g) = resident group (RES_GRP kbs, no DMA in L2),
        # ("slab", kb0) = streamed slab of SG kbs, ("tail", kb0) = L1-tail
        # slab reused in L2 without DMA.
        s_units = list(range(b_res, nkb, SG))
        tail_units = s_units[len(s_units) - ring1:]
        res_units = [("res", g) for g in range(n_res_grp)]

        # L1: residents first (PE races ahead while their 2 MiB loads and the
        # stream pipeline fill DMA), streamed units last (DMA-paced finish).
        l1_order = res_units + [("slab", kb0) for kb0 in s_units]

        # L2, per gather chunk (chunk c covers kbs with kb%8 in its band):
        # ring2-prefetched stream units first (release stream slots the
        # moment the chunk's h lands), then remaining stream units
        # Bresenham-interleaved with no-DMA units (residents + reused
        # L1-tail slabs) so PE slot-release tracks DMA pace and resident
        # work fills the DMA-bound phase instead of forming a serial PE tail.
        def unit_kbs(u):
            kind, v = u
            if kind == "res":
                return list(range(v * RES_GRP, (v + 1) * RES_GRP))
            return list(range(v, v + SG))

        band = (nkb // n_cores) // NCHUNK        # kb%8 band width per chunk

        def unit_chunk(u):
            kbs = unit_kbs(u)
            cs = {(kb % (nkb // n_cores)) // band for kb in kbs}
            assert len(cs) == 1, f"unit {u} straddles gather chunks"
            return cs.pop()

        all_units = (res_units
                     + [("tail", kb0) for kb0 in tail_units]
                     + [("slab", kb0) for kb0 in s_units
                        if kb0 not in tail_units])
        l2_order = []
        for c in range(NCHUNK):
            cu = [u for u in all_units if unit_chunk(u) == c]
            streams = [u for u in cu if u[0] == "slab"]
            nodma = [u for u in cu if u[0] != "slab"]
            nodma_kbs = sum(len(unit_kbs(u)) for u in nodma)
            pre = streams[:ring2] if c == 0 else []
            rest_s = streams[len(pre):]
            l2_order.extend(pre)
            ratio = nodma_kbs / max(1, len(rest_s))
            acc, ri = 0.0, 0
            for u in rest_s:
                l2_order.append(u)
                acc += ratio
                while acc >= 1.0 and ri < len(nodma):
                    l2_order.append(nodma[ri])
                    acc -= len(unit_kbs(nodma[ri]))
                    ri += 1
            l2_order.extend(nodma[ri:])

        assert sorted(kb for u in l2_order for kb in unit_kbs(u)) == \
            list(range(nkb))
        # per-chunk restage-DMA order: deliver h m-groups in consumption order
        rest2 = [[] for _ in range(NCHUNK)]
        seen = set()
        for u in l2_order:
            for kb in unit_kbs(u):
                m = kb // (nkb // n_cores)
                c = unit_chunk(u)
                if (m, c) not in seen:
                    seen.add((m, c))
                    rest2[c].append(m)
        unit_orders = [l1_order, l2_order]

        for layer in range(L):
            # ---- big matmul: Y^T[feat, local nodes] = H^T @ adjT ----
            psy = [
                ps.tile([P, cw_y], f32, name=f"psy{c}", tag=f"psy{c}", bufs=1)
                for c in range(nch_y)
            ]
            pos = 0
            for u in unit_orders[layer]:
                kind, v = u
                if kind == "res":
                    if layer == 0:
                        nc.sync.dma_start(
                            out=res_tiles[v][:],
                            in_=adjq[:, v * RES_GRP * 2:(v + 1) * RES_GRP * 2, :],
                        )
                    rhs_t = res_tiles[v]
                    kb0 = v * RES_GRP
                elif kind == "tail" or (layer == 1 and v in l1_tail):
                    rhs_t = l1_tail[v]
                    kb0 = v
                else:
                    pool = adjp if layer == 0 else adjp2
                    slab = pool.tile([P, 2 * SG, rpc], adt, name="slab",
                                     tag=f"slab{layer}")
                    nc.sync.dma_start(
                        out=slab[:], in_=adjq[:, 2 * v:2 * (v + SG), :]
                    )
                    if layer == 0 and v in tail_units:
                        l1_tail[v] = slab
                    rhs_t = slab
                    kb0 = v
                for i, kb in enumerate(unit_kbs(u)):
                    rhs3 = rhs_t[:, 2 * i:2 * i + 2, :]
                    lhsT = hq[:, 2 * kb:2 * kb + 2, :]
                    for c in range(nch_y):
                        nc.tensor.matmul(
                            psy[c][:],
                            lhsT=lhsT,
                            rhs=rhs3[:, :, c * cw_y:(c + 1) * cw_y],
                            start=(pos == 0), stop=(pos == nkb - 1),
                            perf_mode=mybir.MatmulPerfMode.DoubleRow,
                        )
                    pos += 1

            # ---- per-chunk: X^T = elu(Y^T/n + b), then H1 | final out ----
            b_ap = bias_sb[:, layer:layer + 1]
            hm1 = None
            if layer < L - 1:
                hm1 = xp.tile([P, rpc], hdt, name="hm", tag="hm")
            gather_after = {
                (c2 + 1) * (nch_y // NCHUNK) - 1: c2 for c2 in range(NCHUNK)
            }
            for c in range(nch_y):
                e_t = tmp.tile([P, cw_y], f32, name="e_t", tag="e_t")
                nc.scalar.activation(e_t[:], psy[c][:], AF.Exp,
                                     bias=b_ap, scale=inv_n)
                m_t = tmp.tile([P, cw_y], f32, name="m_t", tag="m_t")
                nc.scalar.activation(m_t[:], psy[c][:], AF.Relu,
                                     bias=b_ap, scale=inv_n)
                xc = tmp.tile([P, cw_y], bf, name="xc", tag="xc")
                nc.vector.scalar_tensor_tensor(
                    xc[:], e_t[:], -1.0, m_t[:], op0=ALU.add, op1=ALU.min,
                )
                if layer < L - 1:
                    # H1 chunk: 4 node tiles -> one PSUM bank -> hm1
                    ph = ps.tile([P, 4 * P], f32, name="ph", tag="ph", bufs=2)
                    for t4 in range(4):
                        nc.tensor.matmul(
                            ph[:, t4 * P:(t4 + 1) * P],
                            lhsT=xc[:, t4 * P:(t4 + 1) * P],
                            rhs=wbd_sb[:, P:2 * P], start=True, stop=True,
                        )
                    nc.vector.tensor_copy(
                        hm1[:, c * cw_y:(c + 1) * cw_y], ph[:])
                    if c in gather_after:
                        c2 = gather_after[c]
                        emit_gather_chunk(hm1, c2, rest2[c2])
                else:
                    # final: out = [ys yu] @ Wl[:128] + za @ Wl[128:] + bl
                    po = ps.tile([P, 4 * OUT], f32, name="po", tag="po",
                                 bufs=2)
                    for t4 in range(4):
                        t = c * 4 + t4
                        osl = slice(t4 * OUT, (t4 + 1) * OUT)
                        nc.tensor.matmul(
                            po[:, osl], lhsT=xc[:, t4 * P:(t4 + 1) * P],
                            rhs=wlsu_sb[:],
                            start=True, stop=False, skip_group_check=True,
                        )
                        nc.tensor.matmul(
                            po[:, osl], lhsT=zat_sb[:, t * P:(t + 1) * P],
                            rhs=wlza_sb[:],
                            start=False, stop=False, skip_group_check=True,
                        )
                        nc.tensor.matmul(
                            po[:, osl], lhsT=ones_sb[:], rhs=blr_sb[:],
                            start=False, stop=True, skip_group_check=True,
                        )
                    ot = tmp.tile([P, 4 * OUT], f32, name="ot", tag="ot")
                    nc.vector.tensor_copy(ot[:], po[:])
                    nc.sync.dma_start(
                        out=outp[c * cw_y:(c + 1) * cw_y, :].rearrange(
                            "(t p) f -> p t f", p=P),
                        in_=ot.rearrange("p (t f) -> p t f", f=OUT),
                    )


def build_full(n_cores=N_CORES, n=N, adj_dtype="fp8", h_dtype="fp8",
               use_doublerow=True, num_devices=None, with_collective=True,
               repeats=1, b_res=B_RES, ring1=RING1, ring2=RING2):
    """Build + compile the full SPMD Bass module (one program, 8 cores)."""
    import concourse.bacc as bacc
    import concourse.mybir as mybir
    import concourse.tile as tile

    dt = mybir.dt
    f32, bf = dt.float32, dt.bfloat16
    adt = dt.float8e4
    rpc = n // n_cores
    if num_devices is None:
        num_devices = n_cores

    nc = bacc.Bacc("TRN2", target_bir_lowering=False, debug=False,
                   num_devices=num_devices)
    ins = {
        "adjt": nc.dram_tensor("adjt", [n, rpc], adt, kind="ExternalInput").ap(),
        "zsut": nc.dram_tensor("zsut", [P, n], bf, kind="ExternalInput").ap(),
        "zat": nc.dram_tensor("zat", [D, rpc], bf, kind="ExternalInput").ap(),
        "wbd": nc.dram_tensor("wbd", [P, L * P], bf, kind="ExternalInput").ap(),
        "wlsu": nc.dram_tensor("wlsu", [P, OUT], bf, kind="ExternalInput").ap(),
        "wlza": nc.dram_tensor("wlza", [D, OUT], bf, kind="ExternalInput").ap(),
        "blr": nc.dram_tensor("blr", [1, OUT], bf, kind="ExternalInput").ap(),
        "bias": nc.dram_tensor("bias", [P, L], f32, kind="ExternalInput").ap(),
    }
    outs = {
        "outp": nc.dram_tensor("outp", [rpc, OUT], f32, kind="ExternalOutput").ap(),
    }
    with tile.TileContext(nc) as tc:
        for _ in range(repeats):
            build_kernel_body(tc, ins, outs, n_cores=n_cores, n=n,
                              with_collective=with_collective, b_res=b_res,
                              ring1=ring1, ring2=ring2)
    nc.compile()
    return nc


def prep_inputs(z, adj, Ws, bs, Wl, bl, n_cores=N_CORES, n=N, adj_dtype="fp8"):
    """Host-side sharding: build the per-core input maps."""
    rpc = n // n_cores
    z = np.asarray(z, dtype=np.float32)
    adj = np.asarray(adj, dtype=np.float32)
    Ws = np.asarray(Ws, dtype=np.float32)
    bs = np.asarray(bs, dtype=np.float32)
    Wl = np.asarray(Wl, dtype=np.float32)
    bl = np.asarray(bl, dtype=np.float32)

    adjt = (adj.T * np.float32(n)).astype(FP8)           # [n, n] in [0,1)
    zsut = np.ascontiguousarray(z[:, :2 * D].T).astype(BF16)  # [128, n]
    zat = np.ascontiguousarray(z[:, 2 * D:].T).astype(BF16)   # [64, n]

    wbd = np.zeros((P, L * P), dtype=np.float32)
    for l in range(L):
        wbd[:D, l * P:l * P + D] = Ws[l]
        wbd[D:, l * P + D:(l + 1) * P] = Ws[l]
    wbd = wbd.astype(BF16)
    bias = np.stack([np.concatenate([bs[l], bs[l]]) for l in range(L)],
                    axis=1).astype(np.float32)           # [128, L]
    wlsu = np.ascontiguousarray(Wl[:2 * D]).astype(BF16)
    wlza = np.ascontiguousarray(Wl[2 * D:]).astype(BF16)
    blr = np.ascontiguousarray(bl.reshape(1, OUT)).astype(BF16)

    in_maps = []
    for m in range(n_cores):
        sl = slice(m * rpc, (m + 1) * rpc)
        in_maps.append({
            "adjt": np.ascontiguousarray(adjt[:, sl]),
            "zsut": zsut,
            "zat": np.ascontiguousarray(zat[:, sl]),
            "wbd": wbd,
            "wlsu": wlsu,
            "wlza": wlza,
            "blr": blr,
            "bias": bias,
        })
    return in_maps


_NC_CACHE = {}
ADJ_DTYPE = "fp8"
H_DTYPE = "fp8"
USE_DOUBLEROW = True


def kernel(z, adj, Ws, bs, Wl, bl):
    """Full-input entry point: shard, run on 8 NeuronCores, gather."""
    from concourse.bass_utils import run_bass_kernel_spmd

    if "nc" not in _NC_CACHE:
        _NC_CACHE["nc"] = build_full()
    nc = _NC_CACHE["nc"]

    in_maps = prep_inputs(z, adj, Ws, bs, Wl, bl)
    res = run_bass_kernel_spmd(nc, in_maps, core_ids=list(range(N_CORES)))
    out = np.concatenate(
        [res.results[m]["outp"] for m in range(N_CORES)], axis=0
    ).astype(np.float32)
    return out


# revision 33
# speedup vs baseline: 1.1858x; 1.0009x over previous
"""Trainium2 Bass kernel for a 2-layer dual-branch GCN (nn_ATACGCN).

reference:
    zs, zu, za = split(z)
    ys = elu(adj @ (elu(zs) @ W0) + b0); ys = elu(adj @ (ys @ W1) + b1)
    yu = elu(adj @ (elu(zu) @ W0) + b0); yu = elu(adj @ (yu @ W1) + b1)
    out = concat(ys, yu, za) @ Wl + bl

Strategy: 1D row-shard of the node dimension across 8 NeuronCores. Both
branches share weights, so they are fused into one 128-wide feature block
(block-diagonal W). Each core computes Y^T = H^T @ adjT in PSUM, streaming
its [16384, 2048] slab of adj^T (fp8, host-scaled by N) from HBM.

The kernel is DMA-bound (adj streaming), so the first B_RES kb-blocks of the
adj slab are kept RESIDENT in SBUF across both GCN layers -- layer 2 only
re-streams the non-resident remainder, reuses L1's last ring slabs straight
from SBUF, and interleaves the no-DMA (resident) work with the re-streamed
work so PE-paced slot release tracks DMA pace. Stage A (input elu + H0) is
computed redundantly on every core (one AllGather is ~30-100 us of ncfw
overhead on HW -- the single inter-layer AllGather is kept, its restage
ordered to deliver h in L2's consumption order). h-path DMAs ride the ACT
HWDGE ring so they never queue behind bulk adj DMAs on the SP ring, and the
gather staging buffers use a (partition, tile) row order so every descriptor
is >=1 KiB-contiguous (128 B descriptors are far below SDMA line rate).

elu(x) is composed as min(exp(x) - 1, max(x, 0)); exp/relu run on ACT with
the fp8 descale (1/n) and layer bias folded into the activation's
scale/bias operands.
"""

import numpy as np
import ml_dtypes

BF16 = ml_dtypes.bfloat16
FP8 = ml_dtypes.float8_e4m3

# Problem constants (hardcoded per harness contract).
N = 16384      # nodes
D = 64         # per-branch width
OUT = 64       # output width
L = 2          # gcn layers
N_CORES = 8
P = 128        # SBUF partitions
RPC = N // N_CORES          # rows (nodes) per core

# Tunables
B_RES = 24                  # resident kb-blocks (of kt//2 = 64); 4 KiB/part each
RES_GRP = 4                 # kb-blocks per resident group DMA (2 MiB DMAs)
SG = 2                      # kb-blocks per streamed slab DMA (1 MiB DMAs)
RING1 = 2                   # L1 streaming slab ring depth (units of SG kbs)
RING2 = 6                   # L2 streaming slab ring depth (fresh tag: prefetch
                            # across the inter-layer collective)
CW_SUB = 512                # stage-A / elu subchunk width
NCHUNK = 1                  # inter-layer AllGather chunks (pipelined)
GATHER_KIND = "AllGather"   # "AllGather" (ring, ~10us/step x 7),
                            # "AllToAll" (input replicated 8x to emulate AG),
                            # "Tree" (3 rounds of pairwise AllGathers)


def build_kernel_body(tc, ins, outs, n_cores=N_CORES, n=N, with_collective=True,
                      b_res=B_RES, ring1=RING1, ring2=RING2):
    """Emit the per-core Tile program (fp8 adj + fp8 H, DoubleRow matmuls).

    ins/outs: dicts name -> bass.AP of the DRAM I/O tensors:
      adjt [n, rpc] fp8 (adj.T * n, host-scaled), zsut [128, rpc] bf16,
      zat [64, rpc] bf16, wbd [128, 2*128] bf16, wlsu [128, 64] bf16,
      wlza [64, 64] bf16, blr [1, 64] bf16, bias [128, 2] f32
      ->  outp [rpc, 64] f32
    """
    import concourse.mybir as mybir

    nc = tc.nc
    dt = mybir.dt
    f32, bf = dt.float32, dt.bfloat16
    AF = mybir.ActivationFunctionType
    ALU = mybir.AluOpType
    adt = dt.float8e4
    hdt = dt.float8e4
    inv_n = 1.0 / n

    rpc = n // n_cores
    kt = n // P                  # 128 k-tiles
    nkb = kt // 2                # 64 kb-blocks (DoubleRow: 2 k-tiles each)
    t_pc = rpc // P              # 16 node tiles per core
    cw_y = 512                   # PSUM chunk width (one f32 bank)
    nch_y = rpc // cw_y          # 4
    n_sub = rpc // CW_SUB        # stage-A subchunks
    assert b_res % RES_GRP == 0
    n_res_grp = b_res // RES_GRP

    adjt = ins["adjt"]
    zsut = ins["zsut"]
    zat = ins["zat"]
    wbd, wlsu, wlza = ins["wbd"], ins["wlsu"], ins["wlza"]
    blr, bias = ins["blr"], ins["bias"]
    outp = outs["outp"]

    # DRAM view: q = global k-tile index (0..127).
    adjq = adjt.rearrange("(q p) m -> p q m", p=P)   # [128, 128, rpc]

    with (
        tc.tile_pool(name="consts", bufs=1) as consts,
        tc.tile_pool(name="respool", bufs=1) as respool,
        tc.tile_pool(name="hpool", bufs=1) as hpool,
        tc.tile_pool(name="adjp", bufs=ring1) as adjp,
        tc.tile_pool(name="adjp2", bufs=ring2) as adjp2,
        tc.tile_pool(name="tmp", bufs=2) as tmp,
        tc.tile_pool(name="xp", bufs=1) as xp,
        tc.tile_pool(name="ps", bufs=1, space="PSUM") as ps,
        tc.tile_pool(name="dram", bufs=1, space="DRAM") as dram,
    ):
        # ---- constants to SBUF ----
        wbd_sb = consts.tile([P, L * P], bf, name="wbd_sb")
        nc.scalar.dma_start(out=wbd_sb[:], in_=wbd[:])
        wlsu_sb = consts.tile([P, OUT], bf, name="wlsu_sb")
        nc.scalar.dma_start(out=wlsu_sb[:], in_=wlsu[:])
        wlza_sb = consts.tile([D, OUT], bf, name="wlza_sb")
        nc.scalar.dma_start(out=wlza_sb[:], in_=wlza[:])
        blr_sb = consts.tile([1, OUT], bf, name="blr_sb")
        nc.scalar.dma_start(out=blr_sb[:], in_=blr[:])
        bias_sb = consts.tile([P, L], f32, name="bias_sb")
        nc.scalar.dma_start(out=bias_sb[:], in_=bias[:])
        zat_sb = consts.tile([D, rpc], bf, name="zat_sb")
        nc.sync.dma_start(out=zat_sb[:], in_=zat[:])
        ones_sb = consts.tile([1, P], bf, name="ones_sb")
        nc.vector.memset(ones_sb[:], 1.0)

        # Persistent H tile (shared between layers; 16 KiB/partition).
        # Layout: h[p, q*128 + f] = H[node q*128+p, f].
        h = hpool.tile([P, n], hdt, name="h", tag="h")
        hq = h.rearrange("p (q f) -> p q f", f=P)

        def emit_gather_chunk(hm, c, rest_order):
            """AllGather chunk c of hm [P, rpc] into h (all cores' blocks).

            g_in rows are ordered (partition, tile-within-chunk) so that the
            g_in write and the h restage run with >=1 KiB-contiguous
            descriptors per partition (128 B descriptors otherwise -- far
            below the 512 B full-rate SDMA minimum).
            """
            wc = rpc // NCHUNK          # hm cols per chunk
            if GATHER_KIND == "Tree":
                # Recursive-doubling AllGather: 3 rounds of pairwise
                # exchanges (1 ring step each) instead of one 7-step ring --
                # the ~10us/step ncfw control floor dominates at this size.
                g_in = dram.tile([wc, P], hdt, name=f"g_in{c}")
                nc.scalar.dma_start(
                    out=g_in.rearrange("(p t) f -> p (t f)", p=P),
                    in_=hm[:, c * wc:(c + 1) * wc],
                )
                rounds = [
                    [[2 * a, 2 * a + 1] for a in range(4)],
                    [[0, 2], [1, 3], [4, 6], [5, 7]],
                    [[0, 4], [1, 5], [2, 6], [3, 7]],
                ]
                cur = g_in
                for r, groups in enumerate(rounds):
                    nxt = dram.tile([wc << (r + 1), P], hdt,
                                    name=f"g_t{c}_{r}")
                    if with_collective and n_cores > 1:
                        nc.gpsimd.collective_compute(
                            "AllGather",
                            mybir.AluOpType.bypass,
                            replica_groups=groups,
                            ins=[cur.opt()],
                            outs=[nxt.opt()],
                        )
                    else:
                        nc.scalar.dma_start(
                            out=nxt[:wc << r, :], in_=cur[:])
                    cur = nxt
                g_out = cur
                gm = g_out.rearrange("(m p w) f -> p m (w f)", m=n_cores,
                                     p=P)
                hv = h.rearrange("p (m c w) -> p m c w", m=n_cores, c=NCHUNK)
                for g in rest_order:
                    nc.scalar.dma_start(
                        out=hv[:, g:g + 1, c, :],
                        in_=gm[:, g:g + 1, :],
                    )
                return
            if GATHER_KIND == "AllToAll":
                # A2A with the input replicated n_cores x emulates AllGather
                # with direct peer sends instead of a 7-step ring.
                g_in = dram.tile([n // NCHUNK, P], hdt, name=f"g_in{c}")
                grep = g_in.rearrange("(m p t) f -> m p (t f)", m=n_cores,
                                      p=P)
                for j in range(n_cores):
                    nc.scalar.dma_start(
                        out=grep[j], in_=hm[:, c * wc:(c + 1) * wc])
            else:
                g_in = dram.tile([wc, P], hdt, name=f"g_in{c}")
                nc.scalar.dma_start(
                    out=g_in.rearrange("(p t) f -> p (t f)", p=P),
                    in_=hm[:, c * wc:(c + 1) * wc],
                )
            if with_collective and n_cores > 1:
                g_out = dram.tile(
                    [n // NCHUNK, P], hdt, name=f"g_out{c}",
                    addr_space="Shared" if GATHER_KIND == "AllGather" else "Local",
                )
                nc.gpsimd.collective_compute(
                    GATHER_KIND,
                    mybir.AluOpType.bypass,
                    replica_groups=[list(range(n_cores))],
                    ins=[g_in.opt()],
                    outs=[g_out.opt()],
                )
            else:
                # cost-model-only path (TimelineSim): same DMA pattern minus
                # the collective. Numerically invalid for other cores' tiles.
                g_out = dram.tile([n // NCHUNK, P], hdt, name=f"g_out{c}")
                nc.scalar.dma_start(out=g_out[:wc, :], in_=g_in[:wc, :])
            # g_out row m*wc/128 ... (m, p, t) holds H[node m*rpc + (c*tc+t)*128 + p].
            gm = g_out.rearrange("(m p w) f -> p m (w f)", m=n_cores, p=P)
            hv = h.rearrange("p (m c w) -> p m c w", m=n_cores, c=NCHUNK)
            for g in rest_order:
                nc.scalar.dma_start(
                    out=hv[:, g:g + 1, c, :],
                    in_=gm[:, g:g + 1, :],
                )

        # ---- stage A (redundant on every core): H0 = elu(zsu) @ W0bd ----
        # Full recompute instead of shard+AllGather: one fewer collective
        # (~36 us on HW) for ~11 us of extra zsut streaming, fully overlapped
        # with the resident-adj loads at startup.
        zbig = n // 8
        for ch in range(n // zbig):
            zch = tmp.tile([P, zbig], bf, name="zch", tag="zch")
            nc.scalar.dma_start(
                out=zch[:], in_=zsut[:, ch * zbig:(ch + 1) * zbig])
            for sc in range(zbig // CW_SUB):
                sl = slice(sc * CW_SUB, (sc + 1) * CW_SUB)
                hsl = slice(ch * zbig + sc * CW_SUB,
                            ch * zbig + (sc + 1) * CW_SUB)
                e_t = tmp.tile([P, CW_SUB], f32, name="e_t", tag="e_t")
                nc.scalar.activation(e_t[:], zch[:, sl], AF.Exp)
                m_t = tmp.tile([P, CW_SUB], f32, name="m_t", tag="m_t")
                nc.scalar.activation(m_t[:], zch[:, sl], AF.Relu)
                x0 = tmp.tile([P, CW_SUB], bf, name="x0", tag="x0")
                nc.vector.scalar_tensor_tensor(
                    x0[:], e_t[:], -1.0, m_t[:], op0=ALU.add, op1=ALU.min
                )
                ph = ps.tile([P, CW_SUB], f32, name="ph", tag="ph", bufs=2)
                for t4 in range(CW_SUB // P):
                    nc.tensor.matmul(
                        ph[:, t4 * P:(t4 + 1) * P],
                        lhsT=x0[:, t4 * P:(t4 + 1) * P], rhs=wbd_sb[:, 0:P],
                        start=True, stop=True,
                    )
                nc.vector.tensor_copy(h[:, hsl], ph[:])

        # ---- resident adj groups (loaded once, used by both layers) ----
        res_tiles = [
            respool.tile([P, RES_GRP * 2, rpc], adt, name=f"res{g}",
                         tag=f"res{g}")
            for g in range(n_res_grp)
        ]
        # L1's last ring1 slab units stay valid in SBUF; L2 reuses them.
        l1_tail = {}     # unit kb0 -> slab tile

        # Work units: ("res", g) = resident group (RES_GRP kbs, no DMA in L2),
        # ("slab", kb0) = streamed slab of SG kbs, ("tail", kb0) = L1-tail
        # slab reused in L2 without DMA.
        s_units = list(range(b_res, nkb, SG))
        tail_units = s_units[len(s_units) - ring1:]
        res_units = [("res", g) for g in range(n_res_grp)]

        # L1: residents first (PE races ahead while their 2 MiB loads and the
        # stream pipeline fill DMA), streamed units last (DMA-paced finish).
        l1_order = res_units + [("slab", kb0) for kb0 in s_units]

        # L2, per gather chunk (chunk c covers kbs with kb%8 in its band):
        # ring2-prefetched stream units first (release stream slots the
        # moment the chunk's h lands), then remaining stream units
        # Bresenham-interleaved with no-DMA units (residents + reused
        # L1-tail slabs) so PE slot-release tracks DMA pace and resident
        # work fills the DMA-bound phase instead of forming a serial PE tail.
        def unit_kbs(u):
            kind, v = u
            if kind == "res":
                return list(range(v * RES_GRP, (v + 1) * RES_GRP))
            return list(range(v, v + SG))

        band = (nkb // n_cores) // NCHUNK        # kb%8 band width per chunk

        def unit_chunk(u):
            kbs = unit_kbs(u)
            cs = {(kb % (nkb // n_cores)) // band for kb in kbs}
            assert len(cs) == 1, f"unit {u} straddles gather chunks"
            return cs.pop()

        all_units = (res_units
                     + [("tail", kb0) for kb0 in tail_units]
                     + [("slab", kb0) for kb0 in s_units
                        if kb0 not in tail_units])
        l2_order = []
        for c in range(NCHUNK):
            cu = [u for u in all_units if unit_chunk(u) == c]
            streams = [u for u in cu if u[0] == "slab"]
            nodma = [u for u in cu if u[0] != "slab"]
            nodma_kbs = sum(len(unit_kbs(u)) for u in nodma)
            pre = streams[:ring2] if c == 0 else []
            rest_s = streams[len(pre):]
            l2_order.extend(pre)
            ratio = nodma_kbs / max(1, len(rest_s))
            acc, ri = 0.0, 0
            for u in rest_s:
                l2_order.append(u)
                acc += ratio
                while acc >= 1.0 and ri < len(nodma):
                    l2_order.append(nodma[ri])
                    acc -= len(unit_kbs(nodma[ri]))
                    ri += 1
            l2_order.extend(nodma[ri:])

        assert sorted(kb for u in l2_order for kb in unit_kbs(u)) == \
            list(range(nkb))
        # per-chunk restage-DMA order: deliver h m-groups in consumption order
        rest2 = [[] for _ in range(NCHUNK)]
        seen = set()
        for u in l2_order:
            for kb in unit_kbs(u):
                m = kb // (nkb // n_cores)
                c = unit_chunk(u)
                if (m, c) not in seen:
                    seen.add((m, c))
                    rest2[c].append(m)
        unit_orders = [l1_order, l2_order]

        for layer in range(L):
            # ---- big matmul: Y^T[feat, local nodes] = H^T @ adjT ----
            psy = [
                ps.tile([P, cw_y], f32, name=f"psy{c}", tag=f"psy{c}", bufs=1)
                for c in range(nch_y)
            ]
            pos = 0
            for u in unit_orders[layer]:
                kind, v = u
                if kind == "res":
                    if layer == 0:
                        nc.sync.dma_start(
                            out=res_tiles[v][:],
                            in_=adjq[:, v * RES_GRP * 2:(v + 1) * RES_GRP * 2, :],
                        )
                    rhs_t = res_tiles[v]
                    kb0 = v * RES_GRP
                elif kind == "tail" or (layer == 1 and v in l1_tail):
                    rhs_t = l1_tail[v]
                    kb0 = v
                else:
                    pool = adjp if layer == 0 else adjp2
                    slab = pool.tile([P, 2 * SG, rpc], adt, name="slab",
                                     tag=f"slab{layer}")
                    nc.sync.dma_start(
                        out=slab[:], in_=adjq[:, 2 * v:2 * (v + SG), :]
                    )
                    if layer == 0 and v in tail_units:
                        l1_tail[v] = slab
                    rhs_t = slab
                    kb0 = v
                for i, kb in enumerate(unit_kbs(u)):
                    rhs3 = rhs_t[:, 2 * i:2 * i + 2, :]
                    lhsT = hq[:, 2 * kb:2 * kb + 2, :]
                    for c in range(nch_y):
                        nc.tensor.matmul(
                            psy[c][:],
                            lhsT=lhsT,
                            rhs=rhs3[:, :, c * cw_y:(c + 1) * cw_y],
                            start=(pos == 0), stop=(pos == nkb - 1),
                            perf_mode=mybir.MatmulPerfMode.DoubleRow,
                        )
                    pos += 1

            # ---- per-chunk: X^T = elu(Y^T/n + b), then H1 | final out ----
            b_ap = bias_sb[:, layer:layer + 1]
            hm1 = None
            if layer < L - 1:
                hm1 = xp.tile([P, rpc], hdt, name="hm", tag="hm")
            gather_after = {
                (c2 + 1) * (nch_y // NCHUNK) - 1: c2 for c2 in range(NCHUNK)
            }
            for c in range(nch_y):
                e_t = tmp.tile([P, cw_y], f32, name="e_t", tag="e_t")
                nc.scalar.activation(e_t[:], psy[c][:], AF.Exp,
                                     bias=b_ap, scale=inv_n)
                m_t = tmp.tile([P, cw_y], f32, name="m_t", tag="m_t")
                nc.scalar.activation(m_t[:], psy[c][:], AF.Relu,
                                     bias=b_ap, scale=inv_n)
                xc = tmp.tile([P, cw_y], bf, name="xc", tag="xc")
                nc.vector.scalar_tensor_tensor(
                    xc[:], e_t[:], -1.0, m_t[:], op0=ALU.add, op1=ALU.min,
                )
                if layer < L - 1:
                    # H1 chunk: 4 node tiles -> one PSUM bank -> hm1
                    ph = ps.tile([P, 4 * P], f32, name="ph", tag="ph", bufs=2)
                    for t4 in range(4):
                        nc.tensor.matmul(
                            ph[:, t4 * P:(t4 + 1) * P],
                            lhsT=xc[:, t4 * P:(t4 + 1) * P],
                            rhs=wbd_sb[:, P:2 * P], start=True, stop=True,
                        )
                    nc.vector.tensor_copy(
                        hm1[:, c * cw_y:(c + 1) * cw_y], ph[:])
                    if c in gather_after:
                        c2 = gather_after[c]
                        emit_gather_chunk(hm1, c2, rest2[c2])
                else:
                    # final: out = [ys yu] @ Wl[:128] + za @ Wl[128:] + bl
                    po = ps.tile([P, 4 * OUT], f32, name="po", tag="po",
                                 bufs=2)
                    for t4 in range(4):
                        t = c * 4 + t4
                        osl = slice(t4 * OUT, (t4 + 1) * OUT)
                        nc.tensor.matmul(
                            po[:, osl], lhsT=xc[:, t4 * P:(t4 + 1) * P],
                            rhs=wlsu_sb[:],
                            start=True, stop=False, skip_group_check=True,
                        )
                        nc.tensor.matmul(
                            po[:, osl], lhsT=zat_sb[:, t * P:(t + 1) * P],
                            rhs=wlza_sb[:],
                            start=False, stop=False, skip_group_check=True,
                        )
                        nc.tensor.matmul(
                            po[:, osl], lhsT=ones_sb[:], rhs=blr_sb[:],
                            start=False, stop=True, skip_group_check=True,
                        )
                    ot = tmp.tile([P, 4 * OUT], f32, name="ot", tag="ot")
                    nc.vector.tensor_copy(ot[:], po[:])
                    nc.sync.dma_start(
                        out=outp[c * cw_y:(c + 1) * cw_y, :].rearrange(
                            "(t p) f -> p t f", p=P),
                        in_=ot.rearrange("p (t f) -> p t f", f=OUT),
                    )


def build_full(n_cores=N_CORES, n=N, adj_dtype="fp8", h_dtype="fp8",
               use_doublerow=True, num_devices=None, with_collective=True,
               repeats=1, b_res=B_RES, ring1=RING1, ring2=RING2):
    """Build + compile the full SPMD Bass module (one program, 8 cores)."""
    import concourse.bacc as bacc
    import concourse.mybir as mybir
    import concourse.tile as tile

    dt = mybir.dt
    f32, bf = dt.float32, dt.bfloat16
    adt = dt.float8e4
    rpc = n // n_cores
    if num_devices is None:
        num_devices = n_cores

    nc = bacc.Bacc("TRN2", target_bir_lowering=False, debug=False,
                   num_devices=num_devices)
    ins = {
        "adjt": nc.dram_tensor("adjt", [n, rpc], adt, kind="ExternalInput").ap(),
        "zsut": nc.dram_tensor("zsut", [P, n], bf, kind="ExternalInput").ap(),
        "zat": nc.dram_tensor("zat", [D, rpc], bf, kind="ExternalInput").ap(),
        "wbd": nc.dram_tensor("wbd", [P, L * P], bf, kind="ExternalInput").ap(),
        "wlsu": nc.dram_tensor("wlsu", [P, OUT], bf, kind="ExternalInput").ap(),
        "wlza": nc.dram_tensor("wlza", [D, OUT], bf, kind="ExternalInput").ap(),
        "blr": nc.dram_tensor("blr", [1, OUT], bf, kind="ExternalInput").ap(),
        "bias": nc.dram_tensor("bias", [P, L], f32, kind="ExternalInput").ap(),
    }
    outs = {
        "outp": nc.dram_tensor("outp", [rpc, OUT], f32, kind="ExternalOutput").ap(),
    }
    with tile.TileContext(nc) as tc:
        for _ in range(repeats):
            build_kernel_body(tc, ins, outs, n_cores=n_cores, n=n,
                              with_collective=with_collective, b_res=b_res,
                              ring1=ring1, ring2=ring2)
    nc.compile()
    return nc


def prep_inputs(z, adj, Ws, bs, Wl, bl, n_cores=N_CORES, n=N, adj_dtype="fp8"):
    """Host-side sharding: build the per-core input maps."""
    rpc = n // n_cores
    z = np.asarray(z, dtype=np.float32)
    adj = np.asarray(adj, dtype=np.float32)
    Ws = np.asarray(Ws, dtype=np.float32)
    bs = np.asarray(bs, dtype=np.float32)
    Wl = np.asarray(Wl, dtype=np.float32)
    bl = np.asarray(bl, dtype=np.float32)

    adjt = (adj.T * np.float32(n)).astype(FP8)           # [n, n] in [0,1)
    zsut = np.ascontiguousarray(z[:, :2 * D].T).astype(BF16)  # [128, n]
    zat = np.ascontiguousarray(z[:, 2 * D:].T).astype(BF16)   # [64, n]

    wbd = np.zeros((P, L * P), dtype=np.float32)
    for l in range(L):
        wbd[:D, l * P:l * P + D] = Ws[l]
        wbd[D:, l * P + D:(l + 1) * P] = Ws[l]
    wbd = wbd.astype(BF16)
    bias = np.stack([np.concatenate([bs[l], bs[l]]) for l in range(L)],
                    axis=1).astype(np.float32)           # [128, L]
    wlsu = np.ascontiguousarray(Wl[:2 * D]).astype(BF16)
    wlza = np.ascontiguousarray(Wl[2 * D:]).astype(BF16)
    blr = np.ascontiguousarray(bl.reshape(1, OUT)).astype(BF16)

    in_maps = []
    for m in range(n_cores):
        sl = slice(m * rpc, (m + 1) * rpc)
        in_maps.append({
            "adjt": np.ascontiguousarray(adjt[:, sl]),
            "zsut": zsut,
            "zat": np.ascontiguousarray(zat[:, sl]),
            "wbd": wbd,
            "wlsu": wlsu,
            "wlza": wlza,
            "blr": blr,
            "bias": bias,
        })
    return in_maps


_NC_CACHE = {}
ADJ_DTYPE = "fp8"
H_DTYPE = "fp8"
USE_DOUBLEROW = True


def kernel(z, adj, Ws, bs, Wl, bl):
    """Full-input entry point: shard, run on 8 NeuronCores, gather."""
    from concourse.bass_utils import run_bass_kernel_spmd

    if "nc" not in _NC_CACHE:
        _NC_CACHE["nc"] = build_full()
    nc = _NC_CACHE["nc"]

    in_maps = prep_inputs(z, adj, Ws, bs, Wl, bl)
    res = run_bass_kernel_spmd(nc, in_maps, core_ids=list(range(N_CORES)))
    out = np.concatenate(
        [res.results[m]["outp"] for m in range(N_CORES)], axis=0
    ).astype(np.float32)
    return out
